# revision 16
# baseline (speedup 1.0000x reference)
"""DBN-Sigma whitening on 8 trn2 cores — two NEFF executions sharing
residual SBUF state (no collectives).

Program 1 streams X (host-cast bf16, channel-major) into SBUF tensors
pinned at fixed addresses via alloc_sbuf_tensor_at, computes the raw
per-half second moment S2 = sum x x^T on the PE (bf16 transposes +
matmuls, PSUM accumulation), and writes the tiny S2 partials to HBM.
The host then reduces the 8 cores' partials in f64, forms the 16
group sigmas with the exact mean, takes eigh (exact sigma^{-1/2} —
microseconds of host work), folds in weight, and launches program 2,
which finds X STILL RESIDENT in SBUF (same fixed addresses; SBUF is
not scrubbed between NEFF executions), whitens from it with bf16
matmuls, applies the shift during wide 2-PSUM-bank V/S moves, and
streams the output out in bf16.

This removes: the second read of X (25.7MB f32 in the 2-launch f32
baseline), the f32 output write (25.7MB), and the on-device AllReduce
of the fused variant (whose Comms engine has a ~70us fixed startup in
this environment).  HBM traffic/core: 12.85MB in + 12.85MB out.
"""

import numpy as np
import ml_dtypes
import concourse.bass as bass
import concourse.bacc as bacc
import concourse.mybir as mybir
import concourse.tile as tile
from concourse.bass_utils import run_bass_kernel_spmd

N_CORES = 8
N, C, H, W = 64, 256, 56, 56
HW = H * W
NL = N // N_CORES
G, CG = 16, 16
EPS = 1e-3
M_TOT = N * HW
MC = NL * HW                   # 25088
FP = mybir.dt.float32
BF = mybir.dt.bfloat16

NP_ = NL // 2                  # 4 image pairs (units) per half
FPAIR = 2 * HW                 # 6272
NCH = FPAIR // 128             # 49
NQG = 7                        # 49 = 7*7 transpose chunks per PSUM group

XRES_BYTES = MC * 2            # 50176 B/partition per half (bf16)


def _xres_handles(nc):
    """The two resident X tensors, pinned at the top of SBUF so both
    programs agree on the address and the (left-growing) tile pools
    never reach them."""
    top = nc.sbuf_top - (nc.sbuf_top % 32)       # stay below the reserve
    return [
        nc.alloc_sbuf_tensor_at(f"xres{h}", [128, MC], BF,
                                offset=top - (2 - h) * XRES_BYTES)
        for h in (0, 1)
    ]


def _build_cov():
    nc = bacc.Bacc("TRN2", target_bir_lowering=False, debug=False,
                   num_devices=N_CORES)
    X_d = nc.dram_tensor("X", [C, MC], BF, kind="ExternalInput")
    eye_d = nc.dram_tensor("eye", [128, 128], BF, kind="ExternalInput")
    S2_d = nc.dram_tensor("S2", [2, 128, 128], FP, kind="ExternalOutput")
    X = X_d.ap()
    xres = _xres_handles(nc)

    with tile.TileContext(nc) as tc:
        with (
            tc.tile_pool(name="const", bufs=1) as constp,
            tc.tile_pool(name="xtq", bufs=6) as xtqp,
            tc.tile_pool(name="acc", bufs=1) as accp,
            tc.tile_pool(name="ptp", bufs=3, space="PSUM") as ptp,
            tc.tile_pool(name="cov", bufs=1, space="PSUM") as covp,
        ):
            eye = constp.tile([128, 128], BF)
            nc.sync.dma_start(eye[:], eye_d.ap())
            cov = [covp.tile([128, 128], FP, name=f"cov{h}") for h in (0, 1)]
            s2sb = accp.tile([128, 256], FP)

            qeng = 0
            for h in (0, 1):
                started = False
                for p in range(NP_):
                    xs = xres[h].ap()[:, FPAIR * p:FPAIR * (p + 1)]
                    nc.sync.dma_start(
                        xs, X[128 * h:128 * (h + 1),
                              FPAIR * p:FPAIR * (p + 1)])
                    last_u = (p == NP_ - 1)
                    for q in range(NCH // NQG):
                        pt = ptp.tile([128, NQG * 128], BF, tag="pt")
                        for jj in range(NQG):
                            m0 = 128 * (NQG * q + jj)
                            nc.tensor.transpose(
                                pt[:, 128 * jj:128 * (jj + 1)],
                                xs[:, m0:m0 + 128], eye[:])
                        xtq = xtqp.tile([128, NQG * 128], BF, tag="xtq")
                        if qeng % 2 == 0:
                            nc.vector.tensor_copy(xtq[:], pt[:])
                        else:
                            nc.scalar.activation(
                                xtq[:], pt[:],
                                mybir.ActivationFunctionType.Copy)
                        qeng += 1
                        for jj in range(NQG):
                            sl = xtq[:, 128 * jj:128 * (jj + 1)]
                            nc.tensor.matmul(
                                cov[h][:], sl, sl,
                                start=not started,
                                stop=(last_u and q == NCH // NQG - 1
                                      and jj == NQG - 1),
                                skip_group_check=True)
                            started = True
                nc.vector.tensor_copy(s2sb[:, 128 * h:128 * (h + 1)],
                                      cov[h][:])
                nc.sync.dma_start(S2_d.ap()[h], s2sb[:, 128 * h:128 * (h + 1)])

    nc.compile()
    return nc


def _build_whiten():
    nc = bacc.Bacc("TRN2", target_bir_lowering=False, debug=False,
                   num_devices=N_CORES)
    wm_d = nc.dram_tensor("wm", [2, 128, 128], BF, kind="ExternalInput")
    sh_d = nc.dram_tensor("sh", [128, 2], FP, kind="ExternalInput")
    Xn_d = nc.dram_tensor("Xn", [C, MC], BF, kind="ExternalOutput")
    Xn = Xn_d.ap()
    xres = _xres_handles(nc)

    # 6272 = 6*1024 + 128: per unit, 12 matmuls of 512 (one PSUM bank
    # each, paired into [128,1024] tiles so each V/S move covers two
    # banks in one instruction) + 1 small 128-col matmul.
    with tile.TileContext(nc) as tc:
        with (
            tc.tile_pool(name="const", bufs=1) as constp,
            tc.tile_pool(name="obuf", bufs=3) as obufp,
            tc.tile_pool(name="wps", bufs=3, space="PSUM") as wpsp,
        ):
            wm = [constp.tile([128, 128], BF, name=f"wm{h}") for h in (0, 1)]
            for h in (0, 1):
                nc.sync.dma_start(wm[h][:], wm_d.ap()[h])
            sh = constp.tile([128, 2], FP)
            nc.sync.dma_start(sh[:], sh_d.ap())

            mv = 0
            for h in (0, 1):
                for p in range(NP_):
                    xs = xres[h].ap()[:, FPAIR * p:FPAIR * (p + 1)]
                    ot = obufp.tile([128, FPAIR], BF, tag="o")
                    for k in range(7):
                        kw = 1024 if k < 6 else 128
                        m0 = 1024 * k
                        ps = wpsp.tile([128, kw], FP, tag="wps")
                        nmm = 2 if kw == 1024 else 1
                        for j in range(nmm):
                            nc.tensor.matmul(
                                ps[:, 512 * j:512 * j + min(512, kw)],
                                wm[h][:], xs[:, m0 + 512 * j:
                                             m0 + 512 * j + min(512, kw)])
                        dst = ot[:, m0:m0 + kw]
                        if mv % 2 == 0:
                            nc.vector.tensor_scalar(
                                dst, ps[:], sh[:, h:h + 1], None,
                                mybir.AluOpType.add)
                        else:
                            nc.scalar.activation(
                                dst, ps[:],
                                mybir.ActivationFunctionType.Identity,
                                bias=sh[:, h:h + 1])
                        mv += 1
                    nc.sync.dma_start(
                        Xn[128 * h:128 * (h + 1), FPAIR * p:FPAIR * (p + 1)],
                        ot[:])

    nc.compile()
    return nc


_PROGS = {}


def _programs():
    if "cov" not in _PROGS:
        _PROGS["cov"] = _build_cov()
        _PROGS["wh"] = _build_whiten()
    return _PROGS["cov"], _PROGS["wh"]


def kernel(X, weight, bias, _return_results=False):
    X = np.asarray(X, dtype=np.float32)
    weight = np.asarray(weight, dtype=np.float32).reshape(C)
    bias = np.asarray(bias, dtype=np.float32).reshape(C)
    nc_cov, nc_wh = _programs()

    Xr = X.reshape(N, C, HW)
    mu = Xr.mean(axis=(0, 2), dtype=np.float64)
    Xc = Xr.transpose(1, 0, 2)
    shards = [np.ascontiguousarray(
        Xc[:, NL * i:NL * (i + 1), :]).reshape(C, MC).astype(ml_dtypes.bfloat16)
        for i in range(N_CORES)]
    eye = np.eye(128, dtype=ml_dtypes.bfloat16)

    res_a = run_bass_kernel_spmd(
        nc_cov, [{"X": s, "eye": eye} for s in shards],
        list(range(N_CORES)))

    s2 = np.zeros((2, 128, 128), np.float64)
    for r in res_a.results:
        s2 += r["S2"].astype(np.float64)

    # exact host-side sigma -> eigh -> wm, weight folded in
    wm_bd = np.zeros((2, 128, 128), np.float64)
    for g in range(G):
        h, o = divmod(g, 128 // CG)
        o *= CG
        mg = mu[CG * g:CG * (g + 1)]
        sg = (s2[h][o:o + CG, o:o + CG] / M_TOT - np.outer(mg, mg)
              + EPS * np.eye(CG))
        lam, u = np.linalg.eigh(sg)
        wm_bd[h][o:o + CG, o:o + CG] = (u / np.sqrt(lam)) @ u.T
    wv = weight.astype(np.float64)
    bv = bias.astype(np.float64)
    # stationary A[d,c] = wm[d,c] * w[c];  shift = b - w * (wm @ mu)
    wmS = np.stack([wm_bd[h] * wv[128 * h:128 * (h + 1)][None, :]
                    for h in (0, 1)]).astype(ml_dtypes.bfloat16)
    shift = bv - wv * np.concatenate([wm_bd[0] @ mu[:128],
                                      wm_bd[1] @ mu[128:]])
    sh_in = np.stack([shift[:128], shift[128:]], axis=1).astype(np.float32)

    res_b = run_bass_kernel_spmd(
        nc_wh, [{"wm": wmS, "sh": sh_in} for _ in range(N_CORES)],
        list(range(N_CORES)))

    out = np.empty((C, N, HW), np.float32)
    for i, r in enumerate(res_b.results):
        out[:, NL * i:NL * (i + 1), :] = \
            r["Xn"].reshape(C, NL, HW).astype(np.float32)
    out = np.ascontiguousarray(out.transpose(1, 0, 2)).reshape(N, C, H, W)
    if _return_results:
        return out, (res_a, res_b)
    return out


# revision 17
# speedup vs baseline: 1.0352x; 1.0352x over previous
"""DBN-Sigma whitening on 8 trn2 cores — two NEFF executions sharing
residual SBUF state (no collectives).

Program 1 streams X (host-cast bf16, channel-major) into SBUF tensors
pinned at fixed addresses via alloc_sbuf_tensor_at, computes the raw
per-half second moment S2 = sum x x^T on the PE (bf16 transposes +
matmuls, PSUM accumulation), and writes the tiny S2 partials to HBM.
The host then reduces the 8 cores' partials in f64, forms the 16
group sigmas with the exact mean, takes eigh (exact sigma^{-1/2} —
microseconds of host work), folds in weight, and launches program 2,
which finds X STILL RESIDENT in SBUF (same fixed addresses; SBUF is
not scrubbed between NEFF executions), whitens from it with bf16
matmuls, applies the shift during wide 2-PSUM-bank V/S moves, and
streams the output out in bf16.

This removes: the second read of X (25.7MB f32 in the 2-launch f32
baseline), the f32 output write (25.7MB), and the on-device AllReduce
of the fused variant (whose Comms engine has a ~70us fixed startup in
this environment).  HBM traffic/core: 12.85MB in + 12.85MB out.
"""

import numpy as np
import ml_dtypes
import concourse.bass as bass
import concourse.bacc as bacc
import concourse.mybir as mybir
import concourse.tile as tile
from concourse.bass_utils import run_bass_kernel_spmd

N_CORES = 8
N, C, H, W = 64, 256, 56, 56
HW = H * W
NL = N // N_CORES
G, CG = 16, 16
EPS = 1e-3
M_TOT = N * HW
MC = NL * HW                   # 25088
FP = mybir.dt.float32
BF = mybir.dt.bfloat16

NP_ = NL // 2                  # 4 image pairs (units) per half
FPAIR = 2 * HW                 # 6272
NCH = FPAIR // 128             # 49
NQG = 7                        # 49 = 7*7 transpose chunks per PSUM group

XRES_BYTES = MC * 2            # 50176 B/partition per half (bf16)


def _xres_handles(nc):
    """The two resident X tensors, pinned at the top of SBUF so both
    programs agree on the address and the (left-growing) tile pools
    never reach them."""
    top = nc.sbuf_top - (nc.sbuf_top % 32)       # stay below the reserve
    return [
        nc.alloc_sbuf_tensor_at(f"xres{h}", [128, MC], BF,
                                offset=top - (2 - h) * XRES_BYTES)
        for h in (0, 1)
    ]


def _build_cov():
    nc = bacc.Bacc("TRN2", target_bir_lowering=False, debug=False,
                   num_devices=N_CORES)
    X_d = nc.dram_tensor("X", [C, MC], BF, kind="ExternalInput")
    eye_d = nc.dram_tensor("eye", [128, 128], BF, kind="ExternalInput")
    S2_d = nc.dram_tensor("S2", [2, 128, 128], FP, kind="ExternalOutput")
    X = X_d.ap()
    xres = _xres_handles(nc)

    with tile.TileContext(nc) as tc:
        with (
            tc.tile_pool(name="const", bufs=1) as constp,
            tc.tile_pool(name="xtq", bufs=6) as xtqp,
            tc.tile_pool(name="xbt", bufs=2) as xbtp,
            tc.tile_pool(name="acc", bufs=1) as accp,
            tc.tile_pool(name="ptp", bufs=3, space="PSUM") as ptp,
            tc.tile_pool(name="cov", bufs=1, space="PSUM") as covp,
        ):
            eye = constp.tile([128, 128], BF)
            nc.sync.dma_start(eye[:], eye_d.ap())
            cov = [covp.tile([128, 128], FP, name=f"cov{h}") for h in (0, 1)]
            s2sb = accp.tile([128, 256], FP)

            qeng = 0
            for h in (0, 1):
                started = False
                for p in range(NP_):
                    xs = xres[h].ap()[:, FPAIR * p:FPAIR * (p + 1)]
                    if h == 0 and p == 0:
                        # quarter-split so the PE's first transposes can
                        # start as soon as the first 1568 columns land
                        for iq in range(4):
                            nc.sync.dma_start(
                                xs[:, 1568 * iq:1568 * (iq + 1)],
                                X[128 * h:128 * (h + 1),
                                  FPAIR * p + 1568 * iq:
                                  FPAIR * p + 1568 * (iq + 1)])
                    else:
                        nc.sync.dma_start(
                            xs, X[128 * h:128 * (h + 1),
                                  FPAIR * p:FPAIR * (p + 1)])
                    last_u = (p == NP_ - 1)
                    if last_u:
                        # last unit of each half: transpose via the DMA
                        # xbar instead of the PE (the queues are idle by
                        # then; saves ~2.6us of PE per unit)
                        xbT = xbtp.tile([128, NCH, 128], BF, tag="xbT")
                        nc.scalar.dma_start_transpose(xbT[:], xs)
                        for j in range(NCH):
                            sl = xbT[:, j, :]
                            nc.tensor.matmul(
                                cov[h][:], sl, sl,
                                start=not started,
                                stop=j == NCH - 1,
                                skip_group_check=True)
                            started = True
                        continue
                    for q in range(NCH // NQG):
                        pt = ptp.tile([128, NQG * 128], BF, tag="pt")
                        for jj in range(NQG):
                            m0 = 128 * (NQG * q + jj)
                            nc.tensor.transpose(
                                pt[:, 128 * jj:128 * (jj + 1)],
                                xs[:, m0:m0 + 128], eye[:])
                        xtq = xtqp.tile([128, NQG * 128], BF, tag="xtq")
                        if qeng % 2 == 0:
                            nc.vector.tensor_copy(xtq[:], pt[:])
                        else:
                            nc.scalar.activation(
                                xtq[:], pt[:],
                                mybir.ActivationFunctionType.Copy)
                        qeng += 1
                        for jj in range(NQG):
                            sl = xtq[:, 128 * jj:128 * (jj + 1)]
                            nc.tensor.matmul(
                                cov[h][:], sl, sl,
                                start=not started,
                                stop=False,
                                skip_group_check=True)
                            started = True
                nc.vector.tensor_copy(s2sb[:, 128 * h:128 * (h + 1)],
                                      cov[h][:])
                nc.sync.dma_start(S2_d.ap()[h], s2sb[:, 128 * h:128 * (h + 1)])

    nc.compile()
    return nc


def _build_whiten():
    nc = bacc.Bacc("TRN2", target_bir_lowering=False, debug=False,
                   num_devices=N_CORES)
    wm_d = nc.dram_tensor("wm", [2, 128, 128], BF, kind="ExternalInput")
    sh_d = nc.dram_tensor("sh", [128, 2], FP, kind="ExternalInput")
    Xn_d = nc.dram_tensor("Xn", [C, MC], BF, kind="ExternalOutput")
    Xn = Xn_d.ap()
    xres = _xres_handles(nc)

    # 6272 = 6*1024 + 128: per unit, 12 matmuls of 512 (one PSUM bank
    # each, paired into [128,1024] tiles so each V/S move covers two
    # banks in one instruction) + 1 small 128-col matmul.
    with tile.TileContext(nc) as tc:
        with (
            tc.tile_pool(name="const", bufs=1) as constp,
            tc.tile_pool(name="obuf", bufs=3) as obufp,
            tc.tile_pool(name="wps", bufs=3, space="PSUM") as wpsp,
        ):
            wm = [constp.tile([128, 128], BF, name=f"wm{h}") for h in (0, 1)]
            for h in (0, 1):
                nc.sync.dma_start(wm[h][:], wm_d.ap()[h])
            sh = constp.tile([128, 2], FP)
            nc.sync.dma_start(sh[:], sh_d.ap())

            mv = 0
            for h in (0, 1):
                for p in range(NP_):
                    xs = xres[h].ap()[:, FPAIR * p:FPAIR * (p + 1)]
                    ot = obufp.tile([128, FPAIR], BF, tag="o")
                    for k in range(7):
                        kw = 1024 if k < 6 else 128
                        m0 = 1024 * k
                        ps = wpsp.tile([128, kw], FP, tag="wps")
                        nmm = 2 if kw == 1024 else 1
                        for j in range(nmm):
                            nc.tensor.matmul(
                                ps[:, 512 * j:512 * j + min(512, kw)],
                                wm[h][:], xs[:, m0 + 512 * j:
                                             m0 + 512 * j + min(512, kw)])
                        dst = ot[:, m0:m0 + kw]
                        if k % 2 == 0 and k < 6:
                            nc.vector.tensor_scalar(
                                dst, ps[:], sh[:, h:h + 1], None,
                                mybir.AluOpType.add)
                        else:
                            nc.scalar.activation(
                                dst, ps[:],
                                mybir.ActivationFunctionType.Identity,
                                bias=sh[:, h:h + 1])
                        mv += 1
                    nc.sync.dma_start(
                        Xn[128 * h:128 * (h + 1), FPAIR * p:FPAIR * (p + 1)],
                        ot[:])

    nc.compile()
    return nc


_PROGS = {}


def _programs():
    if "cov" not in _PROGS:
        _PROGS["cov"] = _build_cov()
        _PROGS["wh"] = _build_whiten()
    return _PROGS["cov"], _PROGS["wh"]


def kernel(X, weight, bias, _return_results=False):
    X = np.asarray(X, dtype=np.float32)
    weight = np.asarray(weight, dtype=np.float32).reshape(C)
    bias = np.asarray(bias, dtype=np.float32).reshape(C)
    nc_cov, nc_wh = _programs()

    Xr = X.reshape(N, C, HW)
    mu = Xr.mean(axis=(0, 2), dtype=np.float64)
    Xc = Xr.transpose(1, 0, 2)
    shards = [np.ascontiguousarray(
        Xc[:, NL * i:NL * (i + 1), :]).reshape(C, MC).astype(ml_dtypes.bfloat16)
        for i in range(N_CORES)]
    eye = np.eye(128, dtype=ml_dtypes.bfloat16)

    res_a = run_bass_kernel_spmd(
        nc_cov, [{"X": s, "eye": eye} for s in shards],
        list(range(N_CORES)))

    s2 = np.zeros((2, 128, 128), np.float64)
    for r in res_a.results:
        s2 += r["S2"].astype(np.float64)

    # exact host-side sigma -> eigh -> wm, weight folded in
    wm_bd = np.zeros((2, 128, 128), np.float64)
    for g in range(G):
        h, o = divmod(g, 128 // CG)
        o *= CG
        mg = mu[CG * g:CG * (g + 1)]
        sg = (s2[h][o:o + CG, o:o + CG] / M_TOT - np.outer(mg, mg)
              + EPS * np.eye(CG))
        lam, u = np.linalg.eigh(sg)
        wm_bd[h][o:o + CG, o:o + CG] = (u / np.sqrt(lam)) @ u.T
    wv = weight.astype(np.float64)
    bv = bias.astype(np.float64)
    # stationary A[d,c] = wm[d,c] * w[c];  shift = b - w * (wm @ mu)
    wmS = np.stack([wm_bd[h] * wv[128 * h:128 * (h + 1)][None, :]
                    for h in (0, 1)]).astype(ml_dtypes.bfloat16)
    shift = bv - wv * np.concatenate([wm_bd[0] @ mu[:128],
                                      wm_bd[1] @ mu[128:]])
    sh_in = np.stack([shift[:128], shift[128:]], axis=1).astype(np.float32)

    res_b = run_bass_kernel_spmd(
        nc_wh, [{"wm": wmS, "sh": sh_in} for _ in range(N_CORES)],
        list(range(N_CORES)))

    out = np.empty((C, N, HW), np.float32)
    for i, r in enumerate(res_b.results):
        out[:, NL * i:NL * (i + 1), :] = \
            r["Xn"].reshape(C, NL, HW).astype(np.float32)
    out = np.ascontiguousarray(out.transpose(1, 0, 2)).reshape(N, C, H, W)
    if _return_results:
        return out, (res_a, res_b)
    return out


# revision 18
# speedup vs baseline: 1.1629x; 1.1233x over previous
"""DBN-Sigma whitening on 8 trn2 cores — two NEFF executions sharing
residual SBUF state (no collectives).

Program 1 streams X (host-cast bf16, channel-major) into SBUF tensors
pinned at fixed addresses via alloc_sbuf_tensor_at, computes the raw
per-half second moment S2 = sum x x^T on the PE (bf16 transposes +
matmuls, PSUM accumulation), and writes the tiny S2 partials to HBM.
The host then reduces the 8 cores' partials in f64, forms the 16
group sigmas with the exact mean, takes eigh (exact sigma^{-1/2} —
microseconds of host work), folds in weight, and launches program 2,
which finds X STILL RESIDENT in SBUF (same fixed addresses; SBUF is
not scrubbed between NEFF executions), whitens from it with bf16
matmuls, applies the shift during wide 2-PSUM-bank V/S moves, and
streams the output out in bf16.

This removes: the second read of X (25.7MB f32 in the 2-launch f32
baseline), the f32 output write (25.7MB), and the on-device AllReduce
of the fused variant (whose Comms engine has a ~70us fixed startup in
this environment).  HBM traffic/core: 12.85MB in + 12.85MB out.
"""

import numpy as np
import ml_dtypes
import concourse.bass as bass
import concourse.bacc as bacc
import concourse.mybir as mybir
import concourse.tile as tile
from concourse.bass_utils import run_bass_kernel_spmd

N_CORES = 8
N, C, H, W = 64, 256, 56, 56
HW = H * W
NL = N // N_CORES
G, CG = 16, 16
EPS = 1e-3
M_TOT = N * HW
MC = NL * HW                   # 25088
FP = mybir.dt.float32
BF = mybir.dt.bfloat16

NP_ = NL // 2                  # 4 image pairs (units) per half
FPAIR = 2 * HW                 # 6272
NCH = FPAIR // 128             # 49
NQG = 7                        # 49 = 7*7 transpose chunks per PSUM group

XRES_BYTES = MC * 2            # 50176 B/partition per half (bf16)


def _xres_handles(nc):
    """The two resident X tensors, pinned at the top of SBUF so both
    programs agree on the address and the (left-growing) tile pools
    never reach them."""
    top = nc.sbuf_top - (nc.sbuf_top % 32)       # stay below the reserve
    return [
        nc.alloc_sbuf_tensor_at(f"xres{h}", [128, MC], BF,
                                offset=top - (2 - h) * XRES_BYTES)
        for h in (0, 1)
    ]


def _build_cov():
    nc = bacc.Bacc("TRN2", target_bir_lowering=False, debug=False,
                   num_devices=N_CORES)
    X_d = nc.dram_tensor("X", [C, MC], BF, kind="ExternalInput")
    eye_d = nc.dram_tensor("eye", [128, 128], BF, kind="ExternalInput")
    S2_d = nc.dram_tensor("S2", [2, 128, 128], FP, kind="ExternalOutput")
    X = X_d.ap()
    xres = _xres_handles(nc)

    with tile.TileContext(nc) as tc:
        with (
            tc.tile_pool(name="const", bufs=1) as constp,
            tc.tile_pool(name="xtq", bufs=6) as xtqp,
            tc.tile_pool(name="acc", bufs=1) as accp,
            tc.tile_pool(name="ptp", bufs=3, space="PSUM") as ptp,
            tc.tile_pool(name="cov", bufs=1, space="PSUM") as covp,
        ):
            eye = constp.tile([128, 128], BF)
            nc.sync.dma_start(eye[:], eye_d.ap())
            cov = [covp.tile([128, 128], FP, name=f"cov{h}") for h in (0, 1)]
            s2sb = accp.tile([128, 256], FP)

            qeng = 0
            for h in (0, 1):
                started = False
                for p in range(NP_):
                    xs = xres[h].ap()[:, FPAIR * p:FPAIR * (p + 1)]
                    if h == 0 and p == 0:
                        # quarter-split so the PE's first transposes start
                        # as soon as the first 1568 columns land
                        for iq in range(4):
                            nc.sync.dma_start(
                                xs[:, 1568 * iq:1568 * (iq + 1)],
                                X[0:128, 1568 * iq:1568 * (iq + 1)])
                    else:
                        nc.sync.dma_start(
                            xs, X[128 * h:128 * (h + 1),
                                  FPAIR * p:FPAIR * (p + 1)])
                    last_u = (p == NP_ - 1)
                    for q in range(NCH // NQG):
                        pt = ptp.tile([128, NQG * 128], BF, tag="pt")
                        for jj in range(NQG):
                            m0 = 128 * (NQG * q + jj)
                            nc.tensor.transpose(
                                pt[:, 128 * jj:128 * (jj + 1)],
                                xs[:, m0:m0 + 128], eye[:])
                        xtq = xtqp.tile([128, NQG * 128], BF, tag="xtq")
                        if qeng % 2 == 0:
                            nc.vector.tensor_copy(xtq[:], pt[:])
                        else:
                            nc.scalar.activation(
                                xtq[:], pt[:],
                                mybir.ActivationFunctionType.Copy)
                        qeng += 1
                        for jj in range(NQG):
                            sl = xtq[:, 128 * jj:128 * (jj + 1)]
                            nc.tensor.matmul(
                                cov[h][:], sl, sl,
                                start=not started,
                                stop=(last_u and q == NCH // NQG - 1
                                      and jj == NQG - 1),
                                skip_group_check=True)
                            started = True
                nc.vector.tensor_copy(s2sb[:, 128 * h:128 * (h + 1)],
                                      cov[h][:])
                nc.sync.dma_start(S2_d.ap()[h], s2sb[:, 128 * h:128 * (h + 1)])

    nc.compile()
    return nc


def _build_whiten():
    nc = bacc.Bacc("TRN2", target_bir_lowering=False, debug=False,
                   num_devices=N_CORES)
    wm_d = nc.dram_tensor("wm", [2, 128, 128], BF, kind="ExternalInput")
    sh_d = nc.dram_tensor("sh", [128, 2], FP, kind="ExternalInput")
    Xn_d = nc.dram_tensor("Xn", [C, MC], BF, kind="ExternalOutput")
    Xn = Xn_d.ap()
    xres = _xres_handles(nc)

    # 6272 = 6*1024 + 128: per unit, 12 matmuls of 512 (one PSUM bank
    # each, paired into [128,1024] tiles so each V/S move covers two
    # banks in one instruction) + 1 small 128-col matmul.
    with tile.TileContext(nc) as tc:
        with (
            tc.tile_pool(name="const", bufs=1) as constp,
            tc.tile_pool(name="obuf", bufs=3) as obufp,
            tc.tile_pool(name="wps", bufs=3, space="PSUM") as wpsp,
        ):
            wm = [constp.tile([128, 128], BF, name=f"wm{h}") for h in (0, 1)]
            for h in (0, 1):
                nc.sync.dma_start(wm[h][:], wm_d.ap()[h])
            sh = constp.tile([128, 2], FP)
            nc.sync.dma_start(sh[:], sh_d.ap())

            mv = 0
            for h in (0, 1):
                for p in range(NP_):
                    xs = xres[h].ap()[:, FPAIR * p:FPAIR * (p + 1)]
                    ot = obufp.tile([128, FPAIR], BF, tag="o")
                    for k in range(7):
                        kw = 1024 if k < 6 else 128
                        m0 = 1024 * k
                        ps = wpsp.tile([128, kw], FP, tag="wps")
                        nmm = 2 if kw == 1024 else 1
                        for j in range(nmm):
                            nc.tensor.matmul(
                                ps[:, 512 * j:512 * j + min(512, kw)],
                                wm[h][:], xs[:, m0 + 512 * j:
                                             m0 + 512 * j + min(512, kw)])
                        dst = ot[:, m0:m0 + kw]
                        if k % 2 == 0 and k < 6:
                            nc.vector.tensor_scalar(
                                dst, ps[:], sh[:, h:h + 1], None,
                                mybir.AluOpType.add)
                        else:
                            nc.scalar.activation(
                                dst, ps[:],
                                mybir.ActivationFunctionType.Identity,
                                bias=sh[:, h:h + 1])
                        mv += 1
                    nc.sync.dma_start(
                        Xn[128 * h:128 * (h + 1), FPAIR * p:FPAIR * (p + 1)],
                        ot[:])

    nc.compile()
    return nc


_PROGS = {}


def _programs():
    if "cov" not in _PROGS:
        _PROGS["cov"] = _build_cov()
        _PROGS["wh"] = _build_whiten()
    return _PROGS["cov"], _PROGS["wh"]


def kernel(X, weight, bias, _return_results=False):
    X = np.asarray(X, dtype=np.float32)
    weight = np.asarray(weight, dtype=np.float32).reshape(C)
    bias = np.asarray(bias, dtype=np.float32).reshape(C)
    nc_cov, nc_wh = _programs()

    Xr = X.reshape(N, C, HW)
    mu = Xr.mean(axis=(0, 2), dtype=np.float64)
    Xc = Xr.transpose(1, 0, 2)
    shards = [np.ascontiguousarray(
        Xc[:, NL * i:NL * (i + 1), :]).reshape(C, MC).astype(ml_dtypes.bfloat16)
        for i in range(N_CORES)]
    eye = np.eye(128, dtype=ml_dtypes.bfloat16)

    res_a = run_bass_kernel_spmd(
        nc_cov, [{"X": s, "eye": eye} for s in shards],
        list(range(N_CORES)))

    s2 = np.zeros((2, 128, 128), np.float64)
    for r in res_a.results:
        s2 += r["S2"].astype(np.float64)

    # exact host-side sigma -> eigh -> wm, weight folded in
    wm_bd = np.zeros((2, 128, 128), np.float64)
    for g in range(G):
        h, o = divmod(g, 128 // CG)
        o *= CG
        mg = mu[CG * g:CG * (g + 1)]
        sg = (s2[h][o:o + CG, o:o + CG] / M_TOT - np.outer(mg, mg)
              + EPS * np.eye(CG))
        lam, u = np.linalg.eigh(sg)
        wm_bd[h][o:o + CG, o:o + CG] = (u / np.sqrt(lam)) @ u.T
    wv = weight.astype(np.float64)
    bv = bias.astype(np.float64)
    # stationary A[d,c] = wm[d,c] * w[c];  shift = b - w * (wm @ mu)
    wmS = np.stack([wm_bd[h] * wv[128 * h:128 * (h + 1)][None, :]
                    for h in (0, 1)]).astype(ml_dtypes.bfloat16)
    shift = bv - wv * np.concatenate([wm_bd[0] @ mu[:128],
                                      wm_bd[1] @ mu[128:]])
    sh_in = np.stack([shift[:128], shift[128:]], axis=1).astype(np.float32)

    res_b = run_bass_kernel_spmd(
        nc_wh, [{"wm": wmS, "sh": sh_in} for _ in range(N_CORES)],
        list(range(N_CORES)))

    out = np.empty((C, N, HW), np.float32)
    for i, r in enumerate(res_b.results):
        out[:, NL * i:NL * (i + 1), :] = \
            r["Xn"].reshape(C, NL, HW).astype(np.float32)
    out = np.ascontiguousarray(out.transpose(1, 0, 2)).reshape(N, C, H, W)
    if _return_results:
        return out, (res_a, res_b)
    return out
